# revision 50
# baseline (speedup 1.0000x reference)
"""Trainium2 Bass kernel for CustomMHA (B=2, N=2048, D=1024, H=16, fp32).

Sharding: 8 cores = (batch b = core//4) x (head-group g = core%4, 4 heads each).
Each core computes, for its batch and its 4 heads:
    attn_out_heads @ Wout[rows of its heads]  ->  a partial [N, D] output.
Host sums the 4 bf16 partials per batch (Megatron-style row-parallel output).

v4 design (vs v2 baseline, ~20% faster simulated, much faster measured):
  * ONE concatenated bf16 input tensor + ONE bf16 output per core: the
    axon/PJRT per-exec dispatch cost scales with buffer count, and bf16
    output halves the out-DMA.
  * ALL projections run up front in a PE-dense phase (chains rotate through
    all 4 psum steal slots, curated single-queue DMA order, PE p-state
    warm-up dummies) -- background proj fills used to steal the attention
    phase's two psA slots and stall the QK rotation.
  * Attention per (pair, ig) group, jb in 16 key blocks: QK head-pair (row
    tiling: even head rows 0-63, odd rows 64-127), exp on ACT for pT0 and
    a mix of ACT / DVE-Schraudolph for pT1 (9 of 16 jbs; bit-pattern exp
    via tensor_scalar rint to int16, bitcast to bf16; per-prob rms ~1.8%),
    AV lagged LAG blocks with the ones-column denominator trick.
  * Drains: att/den copies at jb1 of the next group (av0's readers first so
    its psum slot frees earliest); reciprocal on DVE; bcr cast on Pool;
    ALL normalizations (one mask2-matmul broadcast + one tensor_tensor per
    pair-half) and ALL wout tiles deferred to the tail, interleaved with
    the final group's drain (wouts in 512-col halves, 6-deep out staging).

PSUM (8 banks): psA 2 x [128,1024] (QK + broadcast/wout steals),
psAV 2 x [128,1024] (the pair's AV accumulators).
"""

import sys

sys.path.insert(0, "/opt/trn_rl_repo")

import numpy as np

import concourse.bass as bass
import concourse.mybir as mybir
import concourse.tile as tile
from concourse import bacc
from concourse.bass_utils import run_bass_kernel_spmd

F32 = mybir.dt.float32
F32R = mybir.dt.float32r
BF16 = mybir.dt.bfloat16
I16 = mybir.dt.int16
EXP = mybir.ActivationFunctionType.Exp

# Schraudolph exp on bf16 bit patterns: bits16 = rint(dot*SCHR_A + SCHR_B),
# bits16 viewed as bf16 ~= exp(dot*0.125).  SCHR_A folds the attention scale:
# 0.125 * 128/ln(2); SCHR_B = 16256 + c with c=-7.5 tuned on the reference
# logit distribution (per-prob rms rel err ~1.8%; end-to-end ~8e-3 at the
# 25% offload ratio used below -- gate is 2e-2).
SCHR_A = 0.125 * 184.66496233
SCHR_B = 16256.0 - 7.5

N = 2048  # sequence length
D = 1024  # model dim
HL = 4    # heads per core
O = HL * 64  # per-core projection width (256)
P = 128
NSLAB = 512          # cols per projection slab
NSLABS = N // NSLAB  # 4
IG = 1024            # attention query-column group
NJB = N // P         # 16 key blocks
DC = D // P          # 8 contraction chunks
LAG = 2              # AV lags QK/exp by this many key blocks


# element offsets into the single concatenated bf16 input tensor
SZW = D * O        # 262144 (wq/wk/wv each)
SZWOUT = O * D     # 262144
SZX = D * N        # 2097152
OFF_WQ = 0
OFF_WK = OFF_WQ + SZW
OFF_WV = OFF_WK + SZW
OFF_WOUT = OFF_WV + SZW
OFF_XQ = OFF_WOUT + SZWOUT
OFF_XKV = OFF_XQ + SZX
TOT_IN = OFF_XKV + SZX


def build():
    nc = bacc.Bacc("TRN2", debug=False, num_devices=8)
    inp = nc.dram_tensor("inp", [TOT_IN], BF16, kind="ExternalInput").ap()
    xqT = inp[OFF_XQ : OFF_XQ + SZX].rearrange("(d n) -> d n", n=N)
    xkvT = inp[OFF_XKV : OFF_XKV + SZX].rearrange("(d n) -> d n", n=N)
    wq = inp[OFF_WQ : OFF_WQ + SZW].rearrange("(d o) -> d o", o=O)
    wk = inp[OFF_WK : OFF_WK + SZW].rearrange("(d o) -> d o", o=O)
    wv = inp[OFF_WV : OFF_WV + SZW].rearrange("(d o) -> d o", o=O)
    wout = inp[OFF_WOUT : OFF_WOUT + SZWOUT].rearrange("(c o) -> c o", o=D)
    out = nc.dram_tensor("out", [N, D], BF16, kind="ExternalOutput").ap()

    with tile.TileContext(nc) as tc, nc.allow_low_precision(reason="bf16 kernel"):
        with (
            tc.tile_pool(name="consts", bufs=1) as consts,
            tc.tile_pool(name="weights", bufs=1) as wpool,
            tc.tile_pool(name="xT", bufs=4) as xTpool,
            tc.tile_pool(name="proj", bufs=1) as projpool,
            tc.tile_pool(name="probs", bufs=8) as probspool,
            tc.tile_pool(name="bc", bufs=4) as bcpool,
            tc.tile_pool(name="den", bufs=1) as denpool,
            tc.tile_pool(name="ostage", bufs=6) as opool,
            tc.tile_pool(name="pstg", bufs=3) as stgpool,
            tc.tile_pool(name="psA", bufs=2, space="PSUM") as psA,
            tc.tile_pool(name="psAV", bufs=2, space="PSUM") as psAV,
        ):
            # ---- constants ----
            onesf = consts.tile([P, P], F32)
            nc.vector.memset(onesf[:], 1.0)
            onesr = consts.tile([P, P], BF16)
            nc.vector.tensor_copy(onesr[:], onesf[:])
            # pat: [1, 0, 0, ...] column pattern for vpo padding halves
            patf = consts.tile([P, 64], F32)
            nc.vector.memset(patf[:, 0:1], 1.0)
            nc.vector.memset(patf[:, 1:64], 0.0)
            pat = consts.tile([P, 64], BF16)
            nc.vector.tensor_copy(pat[:], patf[:])
            # mask2: lhsT that broadcasts den row 64 -> out rows 0-63 and den
            # row 0 -> out rows 64-127, so one matmul + one tensor_tensor
            # normalizes both heads of a pair at once.
            mask2f = consts.tile([P, P], F32)
            nc.vector.memset(mask2f[:], 0.0)
            nc.vector.memset(mask2f[64:65, 0:64], 1.0)
            nc.vector.memset(mask2f[0:1, 64:128], 1.0)
            mask2r = consts.tile([P, P], BF16)
            nc.vector.tensor_copy(mask2r[:], mask2f[:])

            # ---- weights: declared here, DMA'd in curated order below ----
            with nc.named_scope("weights"):
                wkr = wpool.tile([P, DC, O], BF16, tag="wkr")
                wqr = wpool.tile([P, DC, O], BF16, tag="wqr")
                wvr = wpool.tile([P, DC, O], BF16, tag="wvr")
                woutr = wpool.tile([P, 2, D], BF16, tag="woutr")
                wr = {"wq": wqr, "wk": wkr, "wv": wvr}

            # ---- persistent activations ----
            qpT = projpool.tile([P, 2, N], BF16, tag="qpT")
            kpT = projpool.tile([P, 2, N], BF16, tag="kpT")
            vpo = [
                projpool.tile([P, NJB, P], BF16, tag=f"vpo{h}", name=f"vpo{h}")
                for h in range(HL)
            ]
            attT = projpool.tile([P, 2, N], BF16, tag="attT")

            # vpo padding halves: ones column + zeros
            for h in range(HL):
                pad0 = 64 if h % 2 == 0 else 0
                nc.vector.tensor_copy(
                    vpo[h][:, :, pad0 : pad0 + 64],
                    pat[:, None, :].to_broadcast([P, NJB, 64]),
                )

            # den buffer: rows other than 0/64 are never written by the
            # drains; init them to 1.0 once so the reciprocal stays finite
            # (the mask2 broadcast contracts all 128 rows; NaN*0 = NaN).
            deninit = denpool.tile([P, IG], F32, tag="den", name="deninit")
            nc.vector.memset(deninit[:], 1.0)

            _flip = [0]

            def steal_psum():
                _flip[0] ^= 1
                pool, tag = (psAV, "av") if _flip[0] else (psA, "qk")
                return pool.tile([P, IG], F32, tag=tag, name="steal")

            def steal_psA():
                return psA.tile([P, IG], F32, tag="qk", name="steal")

            def emit_slab_dma(chain, s, q=None):
                xin = xkvT if chain == "kv" else xqT
                slab = xTpool.tile([P, DC, NSLAB], BF16, tag="xT", name="slab")
                (q if q is not None else nc.sync).dma_start(
                    slab[:],
                    xin.rearrange("(c p) n -> p c n", p=P)[
                        :, :, s * NSLAB : (s + 1) * NSLAB
                    ],
                )
                return slab

            def steal_pair(width):
                """Two psum tiles for a split-k chain: low rows -> psE, high
                rows -> psO.  Row-disjoint k=64 matmuls co-execute AND
                double-pump on the PE (~3.7x vs one k=128 chain, measured).
                TensorTensor only reads ONE psum input, so psO is staged to
                SBUF on the idle ACT engine before the DVE combine."""
                psE = psA.tile([P, IG], F32, tag="qk", name="stealE")[:, :width]
                psO = psAV.tile([P, IG], F32, tag="av", name="stealO")[:, :width]
                return psE, psO

            def emit_proj_mms(w, slab, ps2, col_slice, w_is_rhs=False):
                psE, psO = ps2
                for dc in range(DC):
                    for ps, r0 in ((psE, 0), (psO, 64)):
                        rr = slice(r0, r0 + 64)
                        if w_is_rhs:
                            lhsT, rhs = slab[rr, dc, col_slice], w[rr, dc, :]
                        else:
                            lhsT, rhs = w[rr, dc, col_slice], slab[rr, dc, :]
                        nc.tensor.matmul(
                            ps[:], lhsT, rhs,
                            start=(dc == 0), stop=(dc == DC - 1),
                        )

            def stage_psO(psO, width):
                stg = stgpool.tile([P, NSLAB], F32, tag="stg", name="stg")
                nc.scalar.copy(stg[:, :width], psO[:])
                return stg

            def emit_qproj_chain(slab, s, oc):
                ps2 = steal_pair(NSLAB)
                emit_proj_mms(wr["wq"], slab, ps2, slice(oc * P, (oc + 1) * P))
                stg = stage_psO(ps2[1], NSLAB)
                nc.vector.tensor_tensor(
                    qpT[:, oc, s * NSLAB : (s + 1) * NSLAB],
                    ps2[0][:], stg[:], mybir.AluOpType.add,
                )

            def emit_kproj_chain(slab, s, oc):
                ps2 = steal_pair(NSLAB)
                emit_proj_mms(wr["wk"], slab, ps2, slice(oc * P, (oc + 1) * P))
                stg = stage_psO(ps2[1], NSLAB)
                nc.vector.tensor_tensor(
                    kpT[:, oc, s * NSLAB : (s + 1) * NSLAB],
                    ps2[0][:], stg[:], mybir.AluOpType.add,
                )

            def emit_vproj_chain(slab, s, ch):
                jb = s * (NSLAB // P) + ch
                ps2 = steal_pair(O)
                emit_proj_mms(
                    wr["wv"], slab, ps2, slice(ch * P, (ch + 1) * P), w_is_rhs=True
                )
                stg = stage_psO(ps2[1], O)
                for h in range(HL):
                    v0 = 0 if h % 2 == 0 else 64
                    nc.vector.tensor_tensor(
                        vpo[h][:, jb, v0 : v0 + 64],
                        ps2[0][:, h * 64 : (h + 1) * 64],
                        stg[:, h * 64 : (h + 1) * 64],
                        mybir.AluOpType.add,
                    )

            def emit_qk_pair(h0, ig, jb, qk0, qk1):
                oc = h0 // 2
                i0 = ig * IG
                for nb in range(IG // 512):
                    for hh, qk in ((h0, qk0), (h0 + 1, qk1)):
                        row0 = (hh % 2) * 64
                        nc.tensor.matmul(
                            qk[:, nb * 512 : (nb + 1) * 512],
                            kpT[row0 : row0 + 64, oc, jb * P : (jb + 1) * P],
                            qpT[
                                row0 : row0 + 64,
                                oc,
                                i0 + nb * 512 : i0 + (nb + 1) * 512,
                            ],
                            start=True,
                            stop=True,
                        )

            def emit_av(h, av, jb, pT):
                for nb in range(IG // 512):
                    nc.tensor.matmul(
                        av[:, nb * 512 : (nb + 1) * 512],
                        vpo[h][:, jb, :],
                        pT[:, nb * 512 : (nb + 1) * 512],
                        start=(jb == 0),
                        stop=(jb == NJB - 1),
                    )

            def drain_den(h, av, den):
                srow = 64 - (h % 2) * 64
                nc.vector.tensor_copy(den[srow : srow + 1, :], av[srow : srow + 1, :])

            def drain_att(h, ig, av, eng):
                """Copy att rows out of PSUM; frees the av slot."""
                vrow0 = (h % 2) * 64
                i0 = ig * IG
                dst = attT[vrow0 : vrow0 + 64, h // 2, i0 : i0 + IG]
                with nc.allow_low_precision(reason="bf16 kernel"):
                    eng(dst, av[vrow0 : vrow0 + 64, :])
                return dst

            def drain_recip_pair(den, bcr=None):
                # One full-partition approx reciprocal covers both heads' den
                # rows (0 and 64) at base partition 0; unused lanes hold
                # garbage that nothing reads.  DVE cost scales with free size,
                # not partitions.  bcr cast copy goes to Pool (SBUF->SBUF) to
                # keep DVE free for the att drains; the tail skips bcr and
                # broadcasts bcf directly via an f32 matmul.
                bcf = denpool.tile([P, IG], F32, tag="bcf", name="bcf")
                nc.vector.reciprocal_approx_fast(bcf[:], den[:])
                if bcr is not None:
                    with nc.allow_low_precision(reason="bf16 kernel"):
                        nc.gpsimd.tensor_copy(bcr[:], bcf[:])
                return bcf

            def drain_norm_pair(pc, ig, bc, half, f32=False):
                """Normalize both heads of pair pc for one 512-col half."""
                i0 = ig * IG
                hs = slice(i0 + half * 512, i0 + (half + 1) * 512)
                bcp = steal_psA()
                nc.tensor.matmul(
                    bcp[:, 0:512],
                    (mask2f if f32 else mask2r)[:],
                    bc[:, half * 512 : (half + 1) * 512],
                    start=True,
                    stop=True,
                )
                dst = attT[:, pc, hs]
                nc.vector.tensor_tensor(
                    dst, dst, bcp[:, 0:512], mybir.AluOpType.mult
                )

            def emit_wout_half(ib, half, early=False):
                """One 512-col half of an output tile (k=128 chain; the
                split-k trick needs a tensor_tensor combine, which only DVE
                can run from PSUM -- too much DVE for the tail)."""
                fin = steal_psA()[:, 0:512]
                for pc in range(2):
                    nc.tensor.matmul(
                        fin[:],
                        attT[:, pc, ib * P : (ib + 1) * P],
                        woutr[:, pc, half * 512 : (half + 1) * 512],
                        start=(pc == 0),
                        stop=(pc == 1),
                    )
                ot = opool.tile([P, 512], BF16, tag="ostage", name="ot")
                with nc.allow_low_precision(reason="bf16 partial output"):
                    # alternate ACT/DVE so the copies pipeline
                    cpf = (
                        nc.scalar.copy
                        if (ib + half) % 2 == 0
                        else nc.vector.tensor_copy
                    )
                    cpf(ot[:], fin[:])
                nc.sync.dma_start(
                    out[ib * P : (ib + 1) * P, half * 512 : (half + 1) * 512], ot[:]
                )

            # ---- emission order ----
            sl = {}
            with nc.named_scope("proj"):
                # Full projection phase up front: PE-dense, all 4 psum steal
                # slots in rotation, and no psA contention with the attention
                # QK/exp pipeline later.  Slab DMAs stream through xTpool
                # (bufs=4); weight DMAs ride the ACT/Pool queues so kv0 leads
                # the SP queue.  kv0 is fetched in two dc-halves so the first
                # kproj matmuls start ~1.5us earlier.
                warm = opool.tile([P, 2], F32, tag="ostage", name="warm")
                nc.scalar.activation(warm[0:1, :], onesf[0:1, 0:2], EXP, scale=0.125)
                # Curated transfer order on the shared DMA bandwidth:
                # kv0a, wk(dc 0-3), wk(dc 4-7), kv0b first on the fast Pool
                # trigger queue, then wv, wq, q0, kv1, q1, kv2, kv3, q2, q3,
                # wout on SP -- critical-path-first.
                kv0 = xTpool.tile([P, DC, NSLAB], BF16, tag="xT", name="slab")
                xkvr = xkvT.rearrange("(c p) n -> p c n", p=P)
                wkr_src = wk.rearrange("(c p) o -> p c o", p=P)
                nc.sync.dma_start(kv0[:, 0 : DC // 2, :], xkvr[:, 0 : DC // 2, 0:NSLAB])
                nc.sync.dma_start(wkr[:, 0 : DC // 2, :], wkr_src[:, 0 : DC // 2, :])
                nc.sync.dma_start(wkr[:, DC // 2 : DC, :], wkr_src[:, DC // 2 : DC, :])
                nc.sync.dma_start(
                    kv0[:, DC // 2 : DC, :], xkvr[:, DC // 2 : DC, 0:NSLAB]
                )
                sl["kv0"] = kv0
                nc.sync.dma_start(wvr[:], wv.rearrange("(c p) o -> p c o", p=P))
                sl["kv1"] = emit_slab_dma("kv", 1)
                sl["kv2"] = emit_slab_dma("kv", 2)
                sl["kv3"] = emit_slab_dma("kv", 3)
                nc.sync.dma_start(wqr[:], wq.rearrange("(c p) o -> p c o", p=P))
                sl["q0"] = emit_slab_dma("q", 0)
                sl["q1"] = emit_slab_dma("q", 1)
                sl["q2"] = emit_slab_dma("q", 2)
                sl["q3"] = emit_slab_dma("q", 3)
                nc.sync.dma_start(woutr[:], wout.rearrange("(c p) o -> p c o", p=P))
                # PE p-state warm-up: dummy matmuls keep the PE busy during
                # the initial DMA wait so the real chains start at full clock.
                warmps = steal_psA()
                for _ in range(32):
                    nc.tensor.matmul(
                        warmps[:, 0:P], onesr[:, 0:P], onesr[:, 0:P],
                        start=True, stop=True,
                    )
                for s in range(NSLABS):
                    emit_kproj_chain(sl[f"kv{s}"], s, 0)
                    emit_kproj_chain(sl[f"kv{s}"], s, 1)
                    for ch in range(NSLAB // P):
                        emit_vproj_chain(sl[f"kv{s}"], s, ch)
                for s in range(NSLABS):
                    emit_qproj_chain(sl[f"q{s}"], s, 0)
                    emit_qproj_chain(sl[f"q{s}"], s, 1)

            with nc.named_scope("attention"):
                # groups: (pair, ig); pair p covers heads (2p, 2p+1)
                groups = [(0, 0), (1, 0), (1, 1), (0, 1)]
                pend_flush = None  # (h0, av0, av1, [(jb, pT0, pT1), ...])
                pend_drain = None  # dict with h0, ig, av0, av1
                norms = []  # deferred (pc, ig, bcr) for the tail

                def F_w(ib, half):
                    return lambda: emit_wout_half(ib, half, early=True)

                # deadline-driven background fills: (gi, jb) -> emitters.
                # Only the ig0 wout halves remain; all projections ran up
                # front.  g1's attT is fully normalized by (2,6).
                _spots = [(2, j) for j in range(8, 16)] + [
                    (3, j) for j in range(1, 9)
                ]
                FILLS = {}

                def fills(gi, jb):
                    for f in FILLS.get((gi, jb), ()):
                        with nc.named_scope("fill"), tc.high_priority(offset=-(10**6)):
                            f()

                for gi, (p, ig) in enumerate(groups):
                    h0 = 2 * p
                    av0 = psAV.tile([P, IG], F32, tag="av", name="av0")
                    av1 = psAV.tile([P, IG], F32, tag="av", name="av1")
                    pend = []  # (jb, pT0, pT1) not yet AV-consumed
                    for jb in range(NJB):
                        if jb == 0 and pend_flush is not None:
                            fh0, fav0, fav1, fpend = pend_flush
                            with tc.high_priority(offset=-30):
                                for fjb, fpT0, fpT1 in fpend:
                                    emit_av(fh0, fav0, fjb, fpT0)
                                    emit_av(fh0 + 1, fav1, fjb, fpT1)
                            pend_flush = None
                        qk0 = psA.tile([P, IG], F32, tag="qk", name="qk0")
                        qk1 = psA.tile([P, IG], F32, tag="qk", name="qk1")
                        emit_qk_pair(h0, ig, jb, qk0, qk1)
                        pT0 = probspool.tile([P, IG], BF16, tag="pT", name="pT0")
                        nc.scalar.activation(pT0[:], qk0[:], EXP, scale=0.125)
                        if gi < 3:
                            # norms are deferred to the tail, so jb4/6 have
                            # no DVE work; only jb0/1 (drain burst) stay ACT
                            offl = (2, 3, 4, 5, 6, 7, 8, 9, 10, 11, 12, 13, 15)
                        else:
                            # keep jb14/15 on ACT so DVE is free for the tail
                            # den/recip chain right after the last AV
                            offl = (2, 3, 4, 5, 6, 7, 8, 9, 10, 11, 12, 13)
                        if jb in offl:
                            # Schraudolph exp on DVE: bf16 bit pattern via
                            # rint to int16; AV reads the tile bitcast bf16.
                            pT1 = probspool.tile([P, IG], I16, tag="pT", name="pT1")
                            nc.vector.tensor_scalar(
                                pT1[:], qk1[:], SCHR_A, SCHR_B,
                                mybir.AluOpType.mult, mybir.AluOpType.add,
                            )
                            pT1ap = pT1[:].bitcast(BF16)
                        else:
                            pT1 = probspool.tile([P, IG], BF16, tag="pT", name="pT1")
                            nc.scalar.activation(pT1[:], qk1[:], EXP, scale=0.125)
                            pT1ap = pT1[:]
                        pend.append((jb, pT0[:], pT1ap))
                        if pend_drain is not None:
                            d = pend_drain
                            if jb == 1:
                                den = denpool.tile([P, IG], F32, tag="den", name="den")
                                bcr = bcpool.tile([P, IG], BF16, tag="bc", name="bc")
                                # av0's readers first so its psum slot frees
                                # for the new group's AV as early as possible
                                drain_att(
                                    d["h0"], d["ig"], d["av0"], nc.vector.tensor_copy
                                )
                                drain_den(d["h0"], d["av0"], den)
                                drain_att(
                                    d["h0"] + 1, d["ig"], d["av1"], nc.vector.tensor_copy
                                )
                                drain_den(d["h0"] + 1, d["av1"], den)
                                drain_recip_pair(den, bcr)
                                d["bcr"] = bcr
                                d["den"] = den
                            elif jb == 4:
                                norms.append((d["h0"] // 2, d["ig"], d["bcr"]))
                                pend_drain = None
                        if len(pend) > LAG:
                            ajb, apT0, apT1 = pend.pop(0)
                            with tc.high_priority(offset=-30):
                                emit_av(h0, av0, ajb, apT0)
                                emit_av(h0 + 1, av1, ajb, apT1)
                        fills(gi, jb)
                    pend_flush = (h0, av0, av1, pend)
                    pend_drain = {"h0": h0, "ig": ig, "av0": av0, "av1": av1}

                # final flush + drain, interleaved with the tail wout tiles:
                # wout ib 8-11 needs only the first 512 cols of ig1, ib 12-15
                # the second 512, so each half of the final drain releases a
                # batch of wout tiles.
                fh0, fav0, fav1, fpend = pend_flush
                for fjb, fpT0, fpT1 in fpend:
                    emit_av(fh0, fav0, fjb, fpT0)
                    emit_av(fh0 + 1, fav1, fjb, fpT1)
                d = pend_drain
                den = denpool.tile([P, IG], F32, tag="den", name="den")
                drain_den(d["h0"], d["av0"], den)
                drain_den(d["h0"] + 1, d["av1"], den)
                bcf = drain_recip_pair(den)
                drain_att(d["h0"], d["ig"], d["av0"], nc.scalar.copy)
                drain_att(d["h0"] + 1, d["ig"], d["av1"], nc.scalar.copy)
                for pc_, ig_, bcr_ in norms:
                    drain_norm_pair(pc_, ig_, bcr_, 0)
                    drain_norm_pair(pc_, ig_, bcr_, 1)
                drain_norm_pair(d["h0"] // 2, d["ig"], bcf, 0, f32=True)
                with nc.named_scope("wout"):
                    for ib in list(range(0, 8)) + [8, 9, 10, 11]:
                        emit_wout_half(ib, 0)
                        emit_wout_half(ib, 1)
                drain_norm_pair(d["h0"] // 2, d["ig"], bcf, 1, f32=True)
                with nc.named_scope("wout"):
                    for ib in range(12, N // P):
                        emit_wout_half(ib, 0)
                        emit_wout_half(ib, 1)

    nc.compile()
    return nc


_NC = None


def _get_nc():
    global _NC
    if _NC is None:
        _NC = build()
    return _NC


def make_in_maps(q, kv, Wq, Wkv, Wout):
    q = np.ascontiguousarray(q, dtype=np.float32)
    kv = np.ascontiguousarray(kv, dtype=np.float32)
    Wq = np.ascontiguousarray(Wq, dtype=np.float32)
    Wkv = np.ascontiguousarray(Wkv, dtype=np.float32)
    Wout = np.ascontiguousarray(Wout, dtype=np.float32)
    import ml_dtypes

    bf16 = ml_dtypes.bfloat16
    qT = [np.ascontiguousarray(q[b].T.astype(bf16)) for b in range(2)]
    kvT = [np.ascontiguousarray(kv[b].T.astype(bf16)) for b in range(2)]
    in_maps = []
    for c in range(8):
        b, g = c // 4, c % 4
        sl = slice(g * O, (g + 1) * O)
        flat = np.concatenate(
            [
                Wq[:, sl].astype(bf16).ravel(),
                Wkv[:, sl].astype(bf16).ravel(),
                Wkv[:, D + g * O : D + (g + 1) * O].astype(bf16).ravel(),
                Wout[sl, :].astype(bf16).ravel(),
                qT[b].ravel(),
                kvT[b].ravel(),
            ]
        )
        in_maps.append({"inp": flat})
    return in_maps


def gather(results):
    out = np.zeros((2, N, D), dtype=np.float32)
    for c in range(8):
        out[c // 4] += results[c]["out"].astype(np.float32)
    return out


def kernel(**inputs):
    nc = _get_nc()
    in_maps = make_in_maps(**inputs)
    res = run_bass_kernel_spmd(nc, in_maps, core_ids=list(range(8)))
    return gather(res.results)


if __name__ == "__main__":
    rng = np.random.default_rng(0)
    ins = {
        "q": rng.standard_normal((2, N, D), dtype=np.float32),
        "kv": rng.standard_normal((2, N, D), dtype=np.float32),
        "Wq": (rng.standard_normal((D, D), dtype=np.float32) / np.sqrt(D)).astype(np.float32),
        "Wkv": (rng.standard_normal((D, 2 * D), dtype=np.float32) / np.sqrt(D)).astype(np.float32),
        "Wout": (rng.standard_normal((D, D), dtype=np.float32) / np.sqrt(D)).astype(np.float32),
    }
    out = kernel(**ins)
    print("ok", out.shape, out.dtype)



# revision 51
# speedup vs baseline: 1.9562x; 1.9562x over previous
"""Trainium2 Bass kernel for CustomMHA (B=2, N=2048, D=1024, H=16, fp32).

Sharding: 8 cores = (batch b = core//4) x (head-group g = core%4, 4 heads each).
Each core computes, for its batch and its 4 heads:
    attn_out_heads @ Wout[rows of its heads]  ->  a partial [N, D] output.
Host sums the 4 bf16 partials per batch (Megatron-style row-parallel output).

v4 design (vs v2 baseline, ~20% faster simulated, much faster measured):
  * ONE concatenated bf16 input tensor + ONE bf16 output per core: the
    axon/PJRT per-exec dispatch cost scales with buffer count, and bf16
    output halves the out-DMA.
  * ALL projections run up front in a PE-dense phase (curated single-queue
    DMA order, PE p-state warm-up dummies).  Each projection chain splits
    its k=128 contraction chunks into row-paired k=64 halves (psE/psO):
    row-disjoint k=64 matmuls co-execute AND double-pump on the PE (~3.7x
    measured vs serial); the halves combine via an ACT psum->SBUF stage +
    one DVE tensor_tensor add (TT reads at most one PSUM input).
  * Attention per (pair, ig) group, jb in 16 key blocks: QK head-pair (row
    tiling: even head rows 0-63, odd rows 64-127), exp on ACT for pT0 and
    a mix of ACT / DVE-Schraudolph for pT1 (9 of 16 jbs; bit-pattern exp
    via tensor_scalar rint to int16, bitcast to bf16; per-prob rms ~1.8%),
    AV lagged LAG blocks with the ones-column denominator trick.
  * Drains: att/den copies at jb1 of the next group (av0's readers first so
    its psum slot frees earliest); reciprocal on DVE; bcr cast on Pool;
    ALL normalizations (one mask2-matmul broadcast + one tensor_tensor per
    pair-half) and ALL wout tiles deferred to the tail, interleaved with
    the final group's drain (wouts in 512-col halves, 6-deep out staging).

PSUM (8 banks): psA 2 x [128,1024] (QK + broadcast/wout steals),
psAV 2 x [128,1024] (the pair's AV accumulators).
"""

import sys

sys.path.insert(0, "/opt/trn_rl_repo")

import numpy as np

import concourse.bass as bass
import concourse.mybir as mybir
import concourse.tile as tile
from concourse import bacc
from concourse.bass_utils import run_bass_kernel_spmd

F32 = mybir.dt.float32
F32R = mybir.dt.float32r
BF16 = mybir.dt.bfloat16
I16 = mybir.dt.int16
EXP = mybir.ActivationFunctionType.Exp

# Schraudolph exp on bf16 bit patterns: bits16 = rint(dot*SCHR_A + SCHR_B),
# bits16 viewed as bf16 ~= exp(dot*0.125).  SCHR_A folds the attention scale:
# 0.125 * 128/ln(2); SCHR_B = 16256 + c with c=-7.5 tuned on the reference
# logit distribution (per-prob rms rel err ~1.8%; end-to-end ~8e-3 at the
# 25% offload ratio used below -- gate is 2e-2).
SCHR_A = 0.125 * 184.66496233
SCHR_B = 16256.0 - 7.5

N = 2048  # sequence length
D = 1024  # model dim
HL = 4    # heads per core
O = HL * 64  # per-core projection width (256)
P = 128
NSLAB = 512          # cols per projection slab
NSLABS = N // NSLAB  # 4
IG = 1024            # attention query-column group
NJB = N // P         # 16 key blocks
DC = D // P          # 8 contraction chunks
LAG = 2              # AV lags QK/exp by this many key blocks


# element offsets into the single concatenated bf16 input tensor
SZW = D * O        # 262144 (wq/wk/wv each)
SZWOUT = O * D     # 262144
SZX = D * N        # 2097152
OFF_WQ = 0
OFF_WK = OFF_WQ + SZW
OFF_WV = OFF_WK + SZW
OFF_WOUT = OFF_WV + SZW
OFF_XQ = OFF_WOUT + SZWOUT
OFF_XKV = OFF_XQ + SZX
TOT_IN = OFF_XKV + SZX


def build():
    nc = bacc.Bacc("TRN2", debug=False, num_devices=8)
    inp = nc.dram_tensor("inp", [TOT_IN], BF16, kind="ExternalInput").ap()
    xqT = inp[OFF_XQ : OFF_XQ + SZX].rearrange("(d n) -> d n", n=N)
    xkvT = inp[OFF_XKV : OFF_XKV + SZX].rearrange("(d n) -> d n", n=N)
    wq = inp[OFF_WQ : OFF_WQ + SZW].rearrange("(d o) -> d o", o=O)
    wk = inp[OFF_WK : OFF_WK + SZW].rearrange("(d o) -> d o", o=O)
    wv = inp[OFF_WV : OFF_WV + SZW].rearrange("(d o) -> d o", o=O)
    wout = inp[OFF_WOUT : OFF_WOUT + SZWOUT].rearrange("(c o) -> c o", o=D)
    out = nc.dram_tensor("out", [N, D], BF16, kind="ExternalOutput").ap()

    with tile.TileContext(nc) as tc, nc.allow_low_precision(reason="bf16 kernel"):
        with (
            tc.tile_pool(name="consts", bufs=1) as consts,
            tc.tile_pool(name="weights", bufs=1) as wpool,
            tc.tile_pool(name="xT", bufs=4) as xTpool,
            tc.tile_pool(name="proj", bufs=1) as projpool,
            tc.tile_pool(name="probs", bufs=8) as probspool,
            tc.tile_pool(name="bc", bufs=4) as bcpool,
            tc.tile_pool(name="den", bufs=1) as denpool,
            tc.tile_pool(name="ostage", bufs=6) as opool,
            tc.tile_pool(name="pstg", bufs=3) as stgpool,
            tc.tile_pool(name="psA", bufs=2, space="PSUM") as psA,
            tc.tile_pool(name="psAV", bufs=2, space="PSUM") as psAV,
        ):
            # ---- constants ----
            onesf = consts.tile([P, P], F32)
            nc.vector.memset(onesf[:], 1.0)
            onesr = consts.tile([P, P], BF16)
            nc.vector.tensor_copy(onesr[:], onesf[:])
            # pat: [1, 0, 0, ...] column pattern for vpo padding halves
            patf = consts.tile([P, 64], F32)
            nc.vector.memset(patf[:, 0:1], 1.0)
            nc.vector.memset(patf[:, 1:64], 0.0)
            pat = consts.tile([P, 64], BF16)
            nc.vector.tensor_copy(pat[:], patf[:])
            # mask2: lhsT that broadcasts den row 64 -> out rows 0-63 and den
            # row 0 -> out rows 64-127, so one matmul + one tensor_tensor
            # normalizes both heads of a pair at once.
            mask2f = consts.tile([P, P], F32)
            nc.vector.memset(mask2f[:], 0.0)
            nc.vector.memset(mask2f[64:65, 0:64], 1.0)
            nc.vector.memset(mask2f[0:1, 64:128], 1.0)
            mask2r = consts.tile([P, P], BF16)
            nc.vector.tensor_copy(mask2r[:], mask2f[:])

            # ---- weights: declared here, DMA'd in curated order below ----
            with nc.named_scope("weights"):
                wkr = wpool.tile([P, DC, O], BF16, tag="wkr")
                wqr = wpool.tile([P, DC, O], BF16, tag="wqr")
                wvr = wpool.tile([P, DC, O], BF16, tag="wvr")
                woutr = wpool.tile([P, 2, D], BF16, tag="woutr")
                wr = {"wq": wqr, "wk": wkr, "wv": wvr}

            # ---- persistent activations ----
            qpT = projpool.tile([P, 2, N], BF16, tag="qpT")
            kpT = projpool.tile([P, 2, N], BF16, tag="kpT")
            vpo = [
                projpool.tile([P, NJB, P], BF16, tag=f"vpo{h}", name=f"vpo{h}")
                for h in range(HL)
            ]
            attT = projpool.tile([P, 2, N], BF16, tag="attT")

            # vpo padding halves: ones column + zeros
            for h in range(HL):
                pad0 = 64 if h % 2 == 0 else 0
                nc.vector.tensor_copy(
                    vpo[h][:, :, pad0 : pad0 + 64],
                    pat[:, None, :].to_broadcast([P, NJB, 64]),
                )

            # den buffer: rows other than 0/64 are never written by the
            # drains; init them to 1.0 once so the reciprocal stays finite
            # (the mask2 broadcast contracts all 128 rows; NaN*0 = NaN).
            deninit = denpool.tile([P, IG], F32, tag="den", name="deninit")
            nc.vector.memset(deninit[:], 1.0)

            _flip = [0]

            def steal_psum():
                _flip[0] ^= 1
                pool, tag = (psAV, "av") if _flip[0] else (psA, "qk")
                return pool.tile([P, IG], F32, tag=tag, name="steal")

            def steal_psA():
                return psA.tile([P, IG], F32, tag="qk", name="steal")

            def emit_slab_dma(chain, s, q=None):
                xin = xkvT if chain == "kv" else xqT
                slab = xTpool.tile([P, DC, NSLAB], BF16, tag="xT", name="slab")
                (q if q is not None else nc.sync).dma_start(
                    slab[:],
                    xin.rearrange("(c p) n -> p c n", p=P)[
                        :, :, s * NSLAB : (s + 1) * NSLAB
                    ],
                )
                return slab

            def steal_pair(width):
                """Two psum tiles for a split-k chain: low rows -> psE, high
                rows -> psO.  Row-disjoint k=64 matmuls co-execute AND
                double-pump on the PE (~3.7x vs one k=128 chain, measured).
                TensorTensor only reads ONE psum input, so psO is staged to
                SBUF on the idle ACT engine before the DVE combine."""
                psE = psA.tile([P, IG], F32, tag="qk", name="stealE")[:, :width]
                psO = psAV.tile([P, IG], F32, tag="av", name="stealO")[:, :width]
                return psE, psO

            def emit_proj_mms(w, slab, ps2, col_slice, w_is_rhs=False):
                psE, psO = ps2
                for dc in range(DC):
                    for ps, r0 in ((psE, 0), (psO, 64)):
                        rr = slice(r0, r0 + 64)
                        if w_is_rhs:
                            lhsT, rhs = slab[rr, dc, col_slice], w[rr, dc, :]
                        else:
                            lhsT, rhs = w[rr, dc, col_slice], slab[rr, dc, :]
                        nc.tensor.matmul(
                            ps[:], lhsT, rhs,
                            start=(dc == 0), stop=(dc == DC - 1),
                        )

            def stage_psO(psO, width):
                stg = stgpool.tile([P, NSLAB], F32, tag="stg", name="stg")
                nc.scalar.copy(stg[:, :width], psO[:])
                return stg

            def emit_qproj_chain(slab, s, oc):
                ps2 = steal_pair(NSLAB)
                emit_proj_mms(wr["wq"], slab, ps2, slice(oc * P, (oc + 1) * P))
                stg = stage_psO(ps2[1], NSLAB)
                nc.vector.tensor_tensor(
                    qpT[:, oc, s * NSLAB : (s + 1) * NSLAB],
                    ps2[0][:], stg[:], mybir.AluOpType.add,
                )

            def emit_kproj_chain(slab, s, oc):
                ps2 = steal_pair(NSLAB)
                emit_proj_mms(wr["wk"], slab, ps2, slice(oc * P, (oc + 1) * P))
                stg = stage_psO(ps2[1], NSLAB)
                nc.vector.tensor_tensor(
                    kpT[:, oc, s * NSLAB : (s + 1) * NSLAB],
                    ps2[0][:], stg[:], mybir.AluOpType.add,
                )

            def emit_vproj_chain(slab, s, ch):
                jb = s * (NSLAB // P) + ch
                ps2 = steal_pair(O)
                emit_proj_mms(
                    wr["wv"], slab, ps2, slice(ch * P, (ch + 1) * P), w_is_rhs=True
                )
                stg = stage_psO(ps2[1], O)
                for h in range(HL):
                    v0 = 0 if h % 2 == 0 else 64
                    nc.vector.tensor_tensor(
                        vpo[h][:, jb, v0 : v0 + 64],
                        ps2[0][:, h * 64 : (h + 1) * 64],
                        stg[:, h * 64 : (h + 1) * 64],
                        mybir.AluOpType.add,
                    )

            def emit_qk_pair(h0, ig, jb, qk0, qk1):
                oc = h0 // 2
                i0 = ig * IG
                for nb in range(IG // 512):
                    for hh, qk in ((h0, qk0), (h0 + 1, qk1)):
                        row0 = (hh % 2) * 64
                        nc.tensor.matmul(
                            qk[:, nb * 512 : (nb + 1) * 512],
                            kpT[row0 : row0 + 64, oc, jb * P : (jb + 1) * P],
                            qpT[
                                row0 : row0 + 64,
                                oc,
                                i0 + nb * 512 : i0 + (nb + 1) * 512,
                            ],
                            start=True,
                            stop=True,
                        )

            def emit_av(h, av, jb, pT):
                for nb in range(IG // 512):
                    nc.tensor.matmul(
                        av[:, nb * 512 : (nb + 1) * 512],
                        vpo[h][:, jb, :],
                        pT[:, nb * 512 : (nb + 1) * 512],
                        start=(jb == 0),
                        stop=(jb == NJB - 1),
                    )

            def drain_den(h, av, den):
                srow = 64 - (h % 2) * 64
                nc.vector.tensor_copy(den[srow : srow + 1, :], av[srow : srow + 1, :])

            def drain_att(h, ig, av, eng):
                """Copy att rows out of PSUM; frees the av slot."""
                vrow0 = (h % 2) * 64
                i0 = ig * IG
                dst = attT[vrow0 : vrow0 + 64, h // 2, i0 : i0 + IG]
                with nc.allow_low_precision(reason="bf16 kernel"):
                    eng(dst, av[vrow0 : vrow0 + 64, :])
                return dst

            def drain_recip_pair(den, bcr=None):
                # One full-partition approx reciprocal covers both heads' den
                # rows (0 and 64) at base partition 0; unused lanes hold
                # garbage that nothing reads.  DVE cost scales with free size,
                # not partitions.  bcr cast copy goes to Pool (SBUF->SBUF) to
                # keep DVE free for the att drains; the tail skips bcr and
                # broadcasts bcf directly via an f32 matmul.
                bcf = denpool.tile([P, IG], F32, tag="bcf", name="bcf")
                nc.vector.reciprocal_approx_fast(bcf[:], den[:])
                if bcr is not None:
                    with nc.allow_low_precision(reason="bf16 kernel"):
                        nc.gpsimd.tensor_copy(bcr[:], bcf[:])
                return bcf

            def drain_norm_pair(pc, ig, bc, half, f32=False):
                """Normalize both heads of pair pc for one 512-col half."""
                i0 = ig * IG
                hs = slice(i0 + half * 512, i0 + (half + 1) * 512)
                bcp = steal_psA()
                nc.tensor.matmul(
                    bcp[:, 0:512],
                    (mask2f if f32 else mask2r)[:],
                    bc[:, half * 512 : (half + 1) * 512],
                    start=True,
                    stop=True,
                )
                dst = attT[:, pc, hs]
                nc.vector.tensor_tensor(
                    dst, dst, bcp[:, 0:512], mybir.AluOpType.mult
                )

            def emit_wout_half(ib, half, early=False):
                """One 512-col half of an output tile (k=128 chain; the
                split-k trick needs a tensor_tensor combine, which only DVE
                can run from PSUM -- too much DVE for the tail)."""
                fin = steal_psA()[:, 0:512]
                for pc in range(2):
                    nc.tensor.matmul(
                        fin[:],
                        attT[:, pc, ib * P : (ib + 1) * P],
                        woutr[:, pc, half * 512 : (half + 1) * 512],
                        start=(pc == 0),
                        stop=(pc == 1),
                    )
                ot = opool.tile([P, 512], BF16, tag="ostage", name="ot")
                with nc.allow_low_precision(reason="bf16 partial output"):
                    # alternate ACT/DVE so the copies pipeline
                    cpf = (
                        nc.scalar.copy
                        if (ib + half) % 2 == 0
                        else nc.vector.tensor_copy
                    )
                    cpf(ot[:], fin[:])
                nc.sync.dma_start(
                    out[ib * P : (ib + 1) * P, half * 512 : (half + 1) * 512], ot[:]
                )

            # ---- emission order ----
            sl = {}
            with nc.named_scope("proj"):
                # Full projection phase up front: PE-dense, all 4 psum steal
                # slots in rotation, and no psA contention with the attention
                # QK/exp pipeline later.  Slab DMAs stream through xTpool
                # (bufs=4); weight DMAs ride the ACT/Pool queues so kv0 leads
                # the SP queue.  kv0 is fetched in two dc-halves so the first
                # kproj matmuls start ~1.5us earlier.
                warm = opool.tile([P, 2], F32, tag="ostage", name="warm")
                nc.scalar.activation(warm[0:1, :], onesf[0:1, 0:2], EXP, scale=0.125)
                # Curated transfer order on the shared DMA bandwidth:
                # kv0a, wk(dc 0-3), wk(dc 4-7), kv0b first on the fast Pool
                # trigger queue, then wv, wq, q0, kv1, q1, kv2, kv3, q2, q3,
                # wout on SP -- critical-path-first.
                kv0 = xTpool.tile([P, DC, NSLAB], BF16, tag="xT", name="slab")
                xkvr = xkvT.rearrange("(c p) n -> p c n", p=P)
                wkr_src = wk.rearrange("(c p) o -> p c o", p=P)
                nc.sync.dma_start(kv0[:, 0 : DC // 2, :], xkvr[:, 0 : DC // 2, 0:NSLAB])
                nc.sync.dma_start(wkr[:, 0 : DC // 2, :], wkr_src[:, 0 : DC // 2, :])
                nc.sync.dma_start(wkr[:, DC // 2 : DC, :], wkr_src[:, DC // 2 : DC, :])
                nc.sync.dma_start(
                    kv0[:, DC // 2 : DC, :], xkvr[:, DC // 2 : DC, 0:NSLAB]
                )
                sl["kv0"] = kv0
                nc.sync.dma_start(wvr[:], wv.rearrange("(c p) o -> p c o", p=P))
                sl["kv1"] = emit_slab_dma("kv", 1)
                sl["kv2"] = emit_slab_dma("kv", 2)
                sl["kv3"] = emit_slab_dma("kv", 3)
                nc.sync.dma_start(wqr[:], wq.rearrange("(c p) o -> p c o", p=P))
                sl["q0"] = emit_slab_dma("q", 0)
                sl["q1"] = emit_slab_dma("q", 1)
                sl["q2"] = emit_slab_dma("q", 2)
                sl["q3"] = emit_slab_dma("q", 3)
                nc.sync.dma_start(woutr[:], wout.rearrange("(c p) o -> p c o", p=P))
                # PE p-state warm-up: dummy matmuls keep the PE busy during
                # the initial DMA wait so the real chains start at full clock.
                warmps = steal_psA()
                for _ in range(32):
                    nc.tensor.matmul(
                        warmps[:, 0:P], onesr[:, 0:P], onesr[:, 0:P],
                        start=True, stop=True,
                    )
                for s in range(NSLABS):
                    emit_kproj_chain(sl[f"kv{s}"], s, 0)
                    emit_kproj_chain(sl[f"kv{s}"], s, 1)
                    for ch in range(NSLAB // P):
                        emit_vproj_chain(sl[f"kv{s}"], s, ch)
                for s in range(NSLABS):
                    emit_qproj_chain(sl[f"q{s}"], s, 0)
                    emit_qproj_chain(sl[f"q{s}"], s, 1)

            with nc.named_scope("attention"):
                # groups: (pair, ig); pair p covers heads (2p, 2p+1)
                groups = [(0, 0), (1, 0), (1, 1), (0, 1)]
                pend_flush = None  # (h0, av0, av1, [(jb, pT0, pT1), ...])
                pend_drain = None  # dict with h0, ig, av0, av1
                norms = []  # deferred (pc, ig, bcr) for the tail

                def F_w(ib, half):
                    return lambda: emit_wout_half(ib, half, early=True)

                # deadline-driven background fills: (gi, jb) -> emitters.
                # Only the ig0 wout halves remain; all projections ran up
                # front.  g1's attT is fully normalized by (2,6).
                _spots = [(2, j) for j in range(8, 16)] + [
                    (3, j) for j in range(1, 9)
                ]
                FILLS = {}

                def fills(gi, jb):
                    for f in FILLS.get((gi, jb), ()):
                        with nc.named_scope("fill"), tc.high_priority(offset=-(10**6)):
                            f()

                for gi, (p, ig) in enumerate(groups):
                    h0 = 2 * p
                    av0 = psAV.tile([P, IG], F32, tag="av", name="av0")
                    av1 = psAV.tile([P, IG], F32, tag="av", name="av1")
                    pend = []  # (jb, pT0, pT1) not yet AV-consumed
                    for jb in range(NJB):
                        if jb == 0 and pend_flush is not None:
                            fh0, fav0, fav1, fpend = pend_flush
                            with tc.high_priority(offset=-30):
                                for fjb, fpT0, fpT1 in fpend:
                                    emit_av(fh0, fav0, fjb, fpT0)
                                    emit_av(fh0 + 1, fav1, fjb, fpT1)
                            pend_flush = None
                        qk0 = psA.tile([P, IG], F32, tag="qk", name="qk0")
                        qk1 = psA.tile([P, IG], F32, tag="qk", name="qk1")
                        emit_qk_pair(h0, ig, jb, qk0, qk1)
                        pT0 = probspool.tile([P, IG], BF16, tag="pT", name="pT0")
                        nc.scalar.activation(pT0[:], qk0[:], EXP, scale=0.125)
                        if gi < 3:
                            # norms are deferred to the tail, so jb4/6 have
                            # no DVE work; only jb0/1 (drain burst) stay ACT
                            offl = (2, 3, 4, 5, 6, 7, 8, 9, 10, 11, 12, 13, 15)
                        else:
                            # keep jb14/15 on ACT so DVE is free for the tail
                            # den/recip chain right after the last AV
                            offl = (2, 3, 4, 5, 6, 7, 8, 9, 10, 11, 12, 13)
                        if jb in offl:
                            # Schraudolph exp on DVE: bf16 bit pattern via
                            # rint to int16; AV reads the tile bitcast bf16.
                            pT1 = probspool.tile([P, IG], I16, tag="pT", name="pT1")
                            nc.vector.tensor_scalar(
                                pT1[:], qk1[:], SCHR_A, SCHR_B,
                                mybir.AluOpType.mult, mybir.AluOpType.add,
                            )
                            pT1ap = pT1[:].bitcast(BF16)
                        else:
                            pT1 = probspool.tile([P, IG], BF16, tag="pT", name="pT1")
                            nc.scalar.activation(pT1[:], qk1[:], EXP, scale=0.125)
                            pT1ap = pT1[:]
                        pend.append((jb, pT0[:], pT1ap))
                        if pend_drain is not None:
                            d = pend_drain
                            if jb == 1:
                                den = denpool.tile([P, IG], F32, tag="den", name="den")
                                bcr = bcpool.tile([P, IG], BF16, tag="bc", name="bc")
                                # av0's readers first so its psum slot frees
                                # for the new group's AV as early as possible
                                drain_att(
                                    d["h0"], d["ig"], d["av0"], nc.vector.tensor_copy
                                )
                                drain_den(d["h0"], d["av0"], den)
                                drain_att(
                                    d["h0"] + 1, d["ig"], d["av1"], nc.vector.tensor_copy
                                )
                                drain_den(d["h0"] + 1, d["av1"], den)
                                drain_recip_pair(den, bcr)
                                d["bcr"] = bcr
                                d["den"] = den
                            elif jb == 4:
                                norms.append((d["h0"] // 2, d["ig"], d["bcr"]))
                                pend_drain = None
                        if len(pend) > LAG:
                            ajb, apT0, apT1 = pend.pop(0)
                            with tc.high_priority(offset=-30):
                                emit_av(h0, av0, ajb, apT0)
                                emit_av(h0 + 1, av1, ajb, apT1)
                        fills(gi, jb)
                    pend_flush = (h0, av0, av1, pend)
                    pend_drain = {"h0": h0, "ig": ig, "av0": av0, "av1": av1}

                # final flush + drain, interleaved with the tail wout tiles:
                # wout ib 8-11 needs only the first 512 cols of ig1, ib 12-15
                # the second 512, so each half of the final drain releases a
                # batch of wout tiles.
                fh0, fav0, fav1, fpend = pend_flush
                for fjb, fpT0, fpT1 in fpend:
                    emit_av(fh0, fav0, fjb, fpT0)
                    emit_av(fh0 + 1, fav1, fjb, fpT1)
                d = pend_drain
                den = denpool.tile([P, IG], F32, tag="den", name="den")
                drain_den(d["h0"], d["av0"], den)
                drain_den(d["h0"] + 1, d["av1"], den)
                bcf = drain_recip_pair(den)
                drain_att(d["h0"], d["ig"], d["av0"], nc.scalar.copy)
                drain_att(d["h0"] + 1, d["ig"], d["av1"], nc.scalar.copy)
                for pc_, ig_, bcr_ in norms:
                    drain_norm_pair(pc_, ig_, bcr_, 0)
                    drain_norm_pair(pc_, ig_, bcr_, 1)
                drain_norm_pair(d["h0"] // 2, d["ig"], bcf, 0, f32=True)
                with nc.named_scope("wout"):
                    for ib in list(range(0, 8)) + [8, 9, 10, 11]:
                        emit_wout_half(ib, 0)
                        emit_wout_half(ib, 1)
                drain_norm_pair(d["h0"] // 2, d["ig"], bcf, 1, f32=True)
                with nc.named_scope("wout"):
                    for ib in range(12, N // P):
                        emit_wout_half(ib, 0)
                        emit_wout_half(ib, 1)

    nc.compile()
    return nc


_NC = None


def _get_nc():
    global _NC
    if _NC is None:
        _NC = build()
    return _NC


def make_in_maps(q, kv, Wq, Wkv, Wout):
    q = np.ascontiguousarray(q, dtype=np.float32)
    kv = np.ascontiguousarray(kv, dtype=np.float32)
    Wq = np.ascontiguousarray(Wq, dtype=np.float32)
    Wkv = np.ascontiguousarray(Wkv, dtype=np.float32)
    Wout = np.ascontiguousarray(Wout, dtype=np.float32)
    import ml_dtypes

    bf16 = ml_dtypes.bfloat16
    qT = [np.ascontiguousarray(q[b].T.astype(bf16)) for b in range(2)]
    kvT = [np.ascontiguousarray(kv[b].T.astype(bf16)) for b in range(2)]
    in_maps = []
    for c in range(8):
        b, g = c // 4, c % 4
        sl = slice(g * O, (g + 1) * O)
        flat = np.concatenate(
            [
                Wq[:, sl].astype(bf16).ravel(),
                Wkv[:, sl].astype(bf16).ravel(),
                Wkv[:, D + g * O : D + (g + 1) * O].astype(bf16).ravel(),
                Wout[sl, :].astype(bf16).ravel(),
                qT[b].ravel(),
                kvT[b].ravel(),
            ]
        )
        in_maps.append({"inp": flat})
    return in_maps


def gather(results):
    out = np.zeros((2, N, D), dtype=np.float32)
    for c in range(8):
        out[c // 4] += results[c]["out"].astype(np.float32)
    return out


def kernel(**inputs):
    nc = _get_nc()
    in_maps = make_in_maps(**inputs)
    res = run_bass_kernel_spmd(nc, in_maps, core_ids=list(range(8)))
    return gather(res.results)


if __name__ == "__main__":
    rng = np.random.default_rng(0)
    ins = {
        "q": rng.standard_normal((2, N, D), dtype=np.float32),
        "kv": rng.standard_normal((2, N, D), dtype=np.float32),
        "Wq": (rng.standard_normal((D, D), dtype=np.float32) / np.sqrt(D)).astype(np.float32),
        "Wkv": (rng.standard_normal((D, 2 * D), dtype=np.float32) / np.sqrt(D)).astype(np.float32),
        "Wout": (rng.standard_normal((D, D), dtype=np.float32) / np.sqrt(D)).astype(np.float32),
    }
    out = kernel(**ins)
    print("ok", out.shape, out.dtype)

